# revision 10
# baseline (speedup 1.0000x reference)
"""CSPAttention Trainium2 kernel: 8-way SPMD (batch x seq-half), no collectives.

Sharding: core = b*2 + half; each core computes 1024 query rows of batch b
against the full 2048-token K/V of that batch.  Weight/activation transposes
and the algebraic folds below are host-side marshalling; all per-token FLOPs
run on device.

Host-side folds (exact algebra, done once in fp32 numpy):
  W_eff = [Wf_L @ Wo_attn | Wf_R @ Wo_conv]   (Wf_L/R = halves of Wf)
  b_eff = bf + Wf_L @ bo_attn + Wf_R @ (Wo_conv @ conv_b + bo_conv)
  qres' = queries + b_eff      (residual rows pre-biased)
so the device graph is:
  conv half  = depthwise3(x_conv)                      (DVE shift-mul-add)
  attn half  = softmax(QK^T/8) V  per head             (PE + ACT exp)
  out        = LN(qres' + W_eff @ [attn; conv])        (PE + DVE/ACT)

Device plan (per core, bf16 operands, fp32 PSUM accumulation):
  Q/K feature-major with per-partition bias drains on DVE; V token-major
  into an augmented [V|1] stationary so the softmax denominator falls out
  of the AV matmul's 65th row.  Scores are computed transposed
  (S.T = K^T Q per head) into [128,1024] PSUM tiles; one ACT Exp per tile
  (ACT does nothing else during attention).  Q/K projection of feature
  chunk oc is immediately followed by heads 2oc/2oc+1 so the exp pipeline
  starts early.  Per head-pair the two 1/denom rows are broadcast to all
  64-feature partitions with a DRAM-bounce DMA hidden under the next
  pair's compute; one TT mult normalizes the pair into the concat buffer.
  W_eff runs activation-stationary into token-major PSUM; residual +
  LayerNorm via bn_stats/bn_aggr + Rsqrt + fused affine_mul_reduce, with
  the beta add on the otherwise-idle GpSimd engine.
"""

import sys

sys.path.insert(0, '/opt/trn_rl_repo')

import numpy as np
import ml_dtypes

import concourse.bass as bass
import concourse.tile as tile
from concourse import bacc, mybir

F32 = mybir.dt.float32
BF16 = mybir.dt.bfloat16
NPBF = ml_dtypes.bfloat16

B, L, D = 4, 2048, 1024
DA = 512          # attention channels
DC = 512          # conv channels
H = 8             # heads
HD = 64           # head dim
N_CORES = 8
TQ = 1024         # query rows per core
TK = 2048         # kv rows per core
NTQ = TQ // 512   # moving tiles of 512
NTK = TK // 512
NQC = TQ // 128   # query chunks
NKC = TK // 128   # kv chunks
NDA = DA // 128
ND = D // 128
LN_EPS = 1e-5

Identity = mybir.ActivationFunctionType.Identity
Exp = mybir.ActivationFunctionType.Exp
Sqrt = mybir.ActivationFunctionType.Sqrt
Square = mybir.ActivationFunctionType.Square
AOp = mybir.AluOpType


def _chunked(t, nch, w, ch0=0, col0=0, ncol=None):
    """DRAM [nch*128, w] chunks ch0.. -> SBUF AP [128, nch, ncol] at col0."""
    if ncol is None:
        ncol = w
    return bass.AP(tensor=t, offset=ch0 * 128 * w + col0,
                   ap=[[w, 128], [128 * w, nch], [1, ncol]])


def _feat_bias(t, nch):
    """DRAM [nch*128] -> SBUF [128, nch] feature-major bias."""
    return bass.AP(tensor=t, offset=0, ap=[[1, 128], [128, nch]])


def _bcast(t, n):
    """DRAM [n] -> [128, n] partition broadcast."""
    return bass.AP(tensor=t, offset=0, ap=[[0, 128], [1, n]])


def _rows(t, w, r0, nr):
    """DRAM [*, w] rows r0:r0+nr -> SBUF [nr, w]."""
    return bass.AP(tensor=t, offset=r0 * w, ap=[[w, nr], [1, w]])


def build_nc(reps: int = 1):
    nc = bacc.Bacc('TRN2', target_bir_lowering=False, debug=False,
                   num_devices=N_CORES)

    def din(name, shape, dt):
        return nc.dram_tensor(name, list(shape), dt, kind='ExternalInput')

    t = {n: din(n, s, dt) for n, s, dt in [
        ('qaT', [DA, TQ], BF16), ('qcT', [DC, TQ + 2], BF16),
        ('qres', [TQ, D], BF16),
        ('kT', [DA, TK], BF16), ('vT', [DA, TK], BF16),
        ('wqT', [DA, DA], BF16), ('wkT', [DA, DA], BF16),
        ('wvT', [DA, DA], BF16), ('wfT', [D, D], BF16),
        ('cw', [DC, 3], F32), ('bq', [DA], F32), ('bk', [DA], F32),
        ('bv', [DA], F32), ('gamma', [D], F32), ('beta', [D], F32)]}
    t['out'] = nc.dram_tensor('out', [TQ, D], F32, kind='ExternalOutput')

    with tile.TileContext(nc) as tc:
        for rep in range(reps):
            _build_rep(nc, tc, rep, t)
    nc.compile()
    return nc


def _build_rep(nc, tc, rep, t):
    R = f'r{rep}_'
    with tc.tile_pool(name=R + 'persist', bufs=1) as pp:
        fT_s = pp.tile([128, ND, TQ], BF16)          # concat features, fm
        q_s = pp.tile([128, NDA, TQ], BF16)
        k_s = pp.tile([128, NDA, TK], BF16)
        vaug = pp.tile([128, NKC, H, 66], BF16)
        o_u = pp.tile([128, NDA, TQ], BF16)          # unnormalized attn out
        r_all = pp.tile([1, H, TQ], BF16)            # per-head 1/denom
        wf_s = pp.tile([128, ND, D], BF16)
        ga_bc = pp.tile([128, D], F32)
        be_bc = pp.tile([128, D], F32)
        eps_s = pp.tile([128, 1], F32)

        with tc.tile_pool(name=R + 'wts', bufs=1) as wp, \
             tc.tile_pool(name=R + 'xin', bufs=1) as xp, \
             tc.tile_pool(name=R + 'dscr', bufs=2, space='DRAM') as dp, \
             tc.tile_pool(name=R + 'nrm', bufs=1) as np_, \
             tc.tile_pool(name=R + 'ps_proj', bufs=2, space='PSUM') as ps_p, \
             tc.tile_pool(name=R + 'pst', bufs=2, space='PSUM') as ps_s, \
             tc.tile_pool(name=R + 'pso', bufs=1, space='PSUM') as ps_o, \
             tc.tile_pool(name=R + 'pwork', bufs=3) as wp2:
            # ---------------- input DMAs (compute-order) ----------------
            cw_s = wp.tile([128, NDA, 3], F32)
            nc.sync.dma_start(out=cw_s, in_=_chunked(t['cw'], NDA, 3))
            qcT_s = xp.tile([128, NDA, TQ + 2], BF16)
            for c in range(NDA):
                nc.sync.dma_start(
                    out=qcT_s[:, c, :],
                    in_=_chunked(t['qcT'], 1, TQ + 2, ch0=c))
            wv_s = wp.tile([128, NDA, DA], BF16)
            nc.sync.dma_start(out=wv_s, in_=_chunked(t['wvT'], NDA, DA))
            bv_bc = wp.tile([128, DA], F32)
            nc.sync.dma_start(out=bv_bc, in_=_bcast(t['bv'], DA))
            xv_s = xp.tile([128, NDA, TK], BF16)
            for tt in range(NTK):
                nc.sync.dma_start(
                    out=xv_s[:, :, tt * 512:(tt + 1) * 512],
                    in_=_chunked(t['vT'], NDA, TK, col0=tt * 512, ncol=512))
            wq_s = wp.tile([128, NDA, DA], BF16)
            nc.sync.dma_start(out=wq_s, in_=_chunked(t['wqT'], NDA, DA))
            wk_s = wp.tile([128, NDA, DA], BF16)
            nc.sync.dma_start(out=wk_s, in_=_chunked(t['wkT'], NDA, DA))
            bq_s = wp.tile([128, NDA], F32)
            nc.sync.dma_start(out=bq_s, in_=_feat_bias(t['bq'], NDA))
            bk_s = wp.tile([128, NDA], F32)
            nc.sync.dma_start(out=bk_s, in_=_feat_bias(t['bk'], NDA))
            xa_s = xp.tile([128, NDA, TQ], BF16)
            nc.sync.dma_start(out=xa_s, in_=_chunked(t['qaT'], NDA, TQ))
            xk_s = xp.tile([128, NDA, TK], BF16)
            for tt in range(NTK):
                nc.sync.dma_start(
                    out=xk_s[:, :, tt * 512:(tt + 1) * 512],
                    in_=_chunked(t['kT'], NDA, TK, col0=tt * 512, ncol=512))
            # prefetch final-phase weights (needed last)
            nc.gpsimd.dma_start(out=wf_s, in_=_chunked(t['wfT'], ND, D))
            nc.gpsimd.dma_start(out=ga_bc, in_=_bcast(t['gamma'], D))
            nc.gpsimd.dma_start(out=be_bc, in_=_bcast(t['beta'], D))
            nc.vector.memset(eps_s, LN_EPS)

            # ---------------- conv branch (DVE only) ----------------
            for c in range(NDA):
                nc.vector.tensor_scalar_mul(
                    fT_s[:, NDA + c, :], qcT_s[:, c, 0:TQ], cw_s[:, c, 0:1])
                for k in (1, 2):
                    nc.vector.scalar_tensor_tensor(
                        fT_s[:, NDA + c, :], qcT_s[:, c, k:k + TQ],
                        cw_s[:, c, k:k + 1], fT_s[:, NDA + c, :],
                        AOp.mult, AOp.add)

            # ---------------- V proj (token-major, augmented ones) ------
            nc.vector.memset(vaug[:, :, :, 64:66], 1.0)
            for tt in range(NTK):
                for kk in range(4):
                    kc = tt * 4 + kk
                    pv = ps_p.tile([128, 512], F32, tag='projp')
                    for dc in range(NDA):
                        nc.tensor.matmul(
                            pv[:, :],
                            xv_s[:, dc, tt * 512 + kk * 128:
                                 tt * 512 + (kk + 1) * 128],
                            wv_s[:, dc, :],
                            start=(dc == 0), stop=(dc == NDA - 1))
                    nc.vector.tensor_tensor(
                        vaug[:, kc, :, 0:64],
                        pv[:].rearrange('p (h x) -> p h x', h=H),
                        bv_bc[:].rearrange('p (h x) -> p h x', h=H),
                        AOp.add)

            # ------- Q/K proj per feature chunk, then its two heads -------
            for oc in range(NDA):
                for tt in range(NTQ):
                    pq = ps_p.tile([128, 512], F32, tag='projp')
                    for dc in range(NDA):
                        nc.tensor.matmul(
                            pq[:, :],
                            wq_s[:, dc, oc * 128:(oc + 1) * 128],
                            xa_s[:, dc, tt * 512:(tt + 1) * 512],
                            start=(dc == 0), stop=(dc == NDA - 1))
                    nc.vector.tensor_scalar_add(
                        q_s[:, oc, tt * 512:(tt + 1) * 512], pq[:, :],
                        bq_s[:, oc:oc + 1])
                for tt in range(NTK):
                    pk = ps_p.tile([128, 512], F32, tag='projp')
                    for dc in range(NDA):
                        nc.tensor.matmul(
                            pk[:, :],
                            wk_s[:, dc, oc * 128:(oc + 1) * 128],
                            xk_s[:, dc, tt * 512:(tt + 1) * 512],
                            start=(dc == 0), stop=(dc == NDA - 1))
                    nc.vector.tensor_scalar_add(
                        k_s[:, oc, tt * 512:(tt + 1) * 512], pk[:, :],
                        bk_s[:, oc:oc + 1])

                # ---------------- two heads of this chunk ----------------
                for h in (2 * oc, 2 * oc + 1):
                    hp = (h % 2) * 64
                    o_ps = ps_o.tile([128, TQ], F32, tag='o')
                    for kc in range(NKC):
                        s_ps = ps_s.tile([128, TQ], F32, tag='s')
                        for tt in range(NTQ):
                            nc.tensor.matmul(
                                s_ps[:, tt * 512:(tt + 1) * 512],
                                k_s[hp:hp + 64, oc, kc * 128:(kc + 1) * 128],
                                q_s[hp:hp + 64, oc, tt * 512:(tt + 1) * 512],
                                start=True, stop=True, skip_group_check=True)
                        p_sb = wp2.tile([128, TQ], BF16, tag='p')
                        nc.scalar.activation(p_sb[:, :], s_ps[:, :], Exp,
                                             scale=0.125)
                        for tt in range(NTQ):
                            nc.tensor.matmul(
                                o_ps[0:65, tt * 512:(tt + 1) * 512],
                                vaug[:, kc, h, 0:65],
                                p_sb[:, tt * 512:(tt + 1) * 512],
                                start=(kc == 0), stop=(kc == NKC - 1),
                                skip_group_check=True)
                    with nc.allow_low_precision(reason='bf16 1/denom'):
                        nc.vector.reciprocal(r_all[0:1, h, :], o_ps[64:65, :])
                    nc.vector.tensor_copy(o_u[hp:hp + 64, oc, :],
                                          o_ps[0:64, :])

                # -------- normalize this pair via one DRAM bounce --------
                rscr = dp.tile([1, 2 * TQ], BF16, tag='rscr')
                nc.sync.dma_start(out=rscr[:, :],
                                  in_=r_all[0:1, 2 * oc:2 * oc + 2, :])
                rb = np_.tile([128, NDA, TQ], BF16, tag='rb')
                for a in range(2):
                    nc.sync.dma_start(
                        out=rb[a * 64:(a + 1) * 64, oc, :],
                        in_=bass.AP(tensor=rscr.tensor,
                                    offset=rscr.offset + a * TQ,
                                    ap=[[0, 64], [1, TQ]]))
                nc.vector.tensor_tensor(
                    fT_s[:, oc, :], o_u[:, oc, :], rb[:, oc, :], AOp.mult)

        # ---------------- W_eff + residual + LayerNorm ----------------
        with tc.tile_pool(name=R + 'lnw', bufs=3) as lp, \
             tc.tile_pool(name=R + 'ps_f', bufs=2, space='PSUM') as ps_f:
            for i in range(NQC):
                pf = ps_f.tile([128, D], F32, tag='f')
                for ot in range(2):
                    for fc in range(ND):
                        nc.tensor.matmul(
                            pf[:, ot * 512:(ot + 1) * 512],
                            fT_s[:, fc, i * 128:(i + 1) * 128],
                            wf_s[:, fc, ot * 512:(ot + 1) * 512],
                            start=(fc == 0), stop=(fc == ND - 1))
                res = lp.tile([128, D], BF16, tag='res')
                nc.gpsimd.dma_start(out=res,
                                    in_=_rows(t['qres'], D, i * 128, 128))
                x_s = lp.tile([128, D], F32, tag='x')
                sx = lp.tile([128, 1], F32, tag='sx')
                nc.vector.scalar_tensor_tensor(x_s[:, :], pf[:, :], 1.0,
                                               res[:, :], AOp.mult, AOp.add,
                                               accum_out=sx[:, :])
                x2 = lp.tile([128, D], F32, tag='x2')
                sq = lp.tile([128, 1], F32, tag='sq')
                nc.scalar.activation(x2[:, :], x_s[:, :], Square,
                                     accum_out=sq[:, :])
                m1 = lp.tile([128, 1], F32, tag='m1')
                nc.vector.tensor_scalar_mul(m1[:, :], sx[:, :], 1.0 / D)
                nvar = lp.tile([128, 1], F32, tag='nv')
                nc.vector.scalar_tensor_tensor(nvar[:, :], m1[:, :], sx[:, :],
                                               sq[:, :], AOp.mult,
                                               AOp.subtract)
                sd = lp.tile([128, 1], F32, tag='sd')
                nc.scalar.activation(sd[:, :], nvar[:, :], Sqrt,
                                     bias=eps_s[:, 0:1], scale=-1.0 / D)
                rstd = lp.tile([128, 1], F32, tag='rs')
                nc.vector.reciprocal(rstd[:, :], sd[:, :])
                nm = lp.tile([128, 1], F32, tag='nm')
                nc.vector.scalar_tensor_tensor(nm[:, :], m1[:, :], -1.0,
                                               rstd[:, :], AOp.mult, AOp.mult)
                t1 = lp.tile([128, D], F32, tag='t1')
                junk = lp.tile([128, 1], F32, tag='jk')
                nc.vector.affine_mul_reduce(t1[:, :], junk[:, :], x_s[:, :],
                                            ga_bc[:, :], rstd[:, 0:1],
                                            nm[:, 0:1])
                o_sb = lp.tile([128, D], F32, tag='ob')
                nc.gpsimd.tensor_tensor(o_sb[:, :], t1[:, :], be_bc[:, :],
                                        AOp.add)
                nc.sync.dma_start(out=_rows(t['out'], D, i * 128, 128),
                                  in_=o_sb[:, :])  # SP queue: tail-only traffic


def make_in_maps(inputs):
    f32 = np.float32
    q = np.ascontiguousarray(np.asarray(inputs['queries'], f32))
    k = np.ascontiguousarray(np.asarray(inputs['keys'], f32))
    v = np.ascontiguousarray(np.asarray(inputs['values'], f32))
    Wf = np.asarray(inputs['Wf'], f32)
    Woa = np.asarray(inputs['Wo_attn'], f32)
    Woc = np.asarray(inputs['Wo_conv'], f32)
    # torch Linear convention: y = x @ W.T + b with W (out, in)
    W_eff = np.concatenate([Wf[:, :DA] @ Woa, Wf[:, DA:] @ Woc], axis=1)
    b_eff = (np.asarray(inputs['bf'], f32)
             + Wf[:, :DA] @ np.asarray(inputs['bo_attn'], f32)
             + Wf[:, DA:] @ (Woc @ np.asarray(inputs['conv_b'], f32)
                             + np.asarray(inputs['bo_conv'], f32)))
    com = {
        'wqT': np.asarray(inputs['Wq'], f32).T.astype(NPBF),
        'wkT': np.asarray(inputs['Wk'], f32).T.astype(NPBF),
        'wvT': np.asarray(inputs['Wv'], f32).T.astype(NPBF),
        'wfT': W_eff.T.astype(NPBF),
        'cw': np.asarray(inputs['conv_w'], f32).reshape(DC, 3),
        'bq': np.asarray(inputs['bq'], f32),
        'bk': np.asarray(inputs['bk'], f32),
        'bv': np.asarray(inputs['bv'], f32),
        'gamma': np.asarray(inputs['gamma'], f32),
        'beta': np.asarray(inputs['beta'], f32),
    }
    com = {n: np.ascontiguousarray(a) for n, a in com.items()}
    in_maps = []
    for core in range(N_CORES):
        b, half = core // 2, core % 2
        r0, r1 = half * TQ, (half + 1) * TQ
        qc = np.zeros((TQ + 2, DC), f32)
        qc[1:TQ + 1] = q[b, r0:r1, DA:]
        if r0 > 0:
            qc[0] = q[b, r0 - 1, DA:]
        if r1 < L:
            qc[TQ + 1] = q[b, r1, DA:]
        m = dict(com)
        m['qaT'] = np.ascontiguousarray(q[b, r0:r1, :DA].T.astype(NPBF))
        m['qcT'] = np.ascontiguousarray(qc.T.astype(NPBF))
        m['qres'] = np.ascontiguousarray((q[b, r0:r1, :] + b_eff).astype(NPBF))
        m['kT'] = np.ascontiguousarray(k[b, :, :DA].T.astype(NPBF))
        m['vT'] = np.ascontiguousarray(v[b, :, :DA].T.astype(NPBF))
        in_maps.append(m)
    return in_maps


_NC_CACHE = {}


def get_nc(reps=1):
    if reps not in _NC_CACHE:
        _NC_CACHE[reps] = build_nc(reps)
    return _NC_CACHE[reps]


def kernel(**inputs):
    from concourse.bass_utils import run_bass_kernel_spmd
    nc = get_nc(1)
    in_maps = make_in_maps(inputs)
    res = run_bass_kernel_spmd(nc, in_maps, core_ids=list(range(N_CORES)))
    out = np.empty((B, L, D), np.float32)
    for core in range(N_CORES):
        b, half = core // 2, core % 2
        out[b, half * TQ:(half + 1) * TQ, :] = res.results[core]['out']
    return out


# revision 46
# speedup vs baseline: 2.2431x; 2.2431x over previous
"""CSPAttention Trainium2 kernel: 8-way SPMD (batch x seq-half), no collectives.

Sharding: core = b*2 + half; each core computes 1024 query rows of batch b
against the full 2048-token K/V of that batch.  Weight/activation transposes
and the algebraic folds below are host-side marshalling; all per-token FLOPs
run on device.

Host-side folds (exact algebra, done once in fp32 numpy):
  W_eff = [Wf_L @ Wo_attn | Wf_R @ Wo_conv]   (Wf_L/R = halves of Wf)
  b_eff = bf + Wf_L @ bo_attn + Wf_R @ (Wo_conv @ conv_b + bo_conv)
  qres' = queries + b_eff      (residual rows pre-biased)
so the device graph is:
  conv half  = depthwise3(x_conv)                      (DVE shift-mul-add)
  attn half  = softmax(QK^T/8) V  per head             (PE + ACT exp)
  out        = LN(qres' + W_eff @ [attn; conv])        (PE + DVE/ACT)

Device plan (per core, bf16 operands, fp32 PSUM accumulation):
  Q/K feature-major drains with per-partition bias on ACT; V token-major
  into an augmented [V|1] stationary so the softmax denominator falls out
  of the AV matmul's 65th row.  Scores are computed transposed
  (S.T = K^T Q per head) into [128,1024] PSUM tiles; one Exp per tile.
  Per-head denominators are collected, reciprocal'd, and broadcast to all
  feature partitions with a single DRAM-bounce DMA; one TT mult per chunk
  normalizes all heads at once.  W_eff runs activation-stationary into
  token-major PSUM; residual + LayerNorm via bn_stats/bn_aggr.
"""

import sys

sys.path.insert(0, '/opt/trn_rl_repo')

import numpy as np
import ml_dtypes

import concourse.bass as bass
import concourse.tile as tile
from concourse import bacc, mybir

F32 = mybir.dt.float32
BF16 = mybir.dt.bfloat16
NPBF = ml_dtypes.bfloat16

B, L, D = 4, 2048, 1024
DA = 512          # attention channels
DC = 512          # conv channels
H = 8             # heads
HD = 64           # head dim
N_CORES = 8
TQ = 1024         # query rows per core
TK = 2048         # kv rows per core
NTQ = TQ // 512   # moving tiles of 512
NTK = TK // 512
NQC = TQ // 128   # query chunks
NKC = TK // 128   # kv chunks
NDA = DA // 128
ND = D // 128
LN_EPS = 1e-5

Identity = mybir.ActivationFunctionType.Identity
Exp = mybir.ActivationFunctionType.Exp
Sqrt = mybir.ActivationFunctionType.Sqrt
AOp = mybir.AluOpType


def _chunked(t, nch, w, col0=0, ncol=None):
    """DRAM [nch*128, w] -> SBUF-layout AP [128, nch, ncol] starting at col0."""
    if ncol is None:
        ncol = w
    return bass.AP(tensor=t, offset=col0,
                   ap=[[w, 128], [128 * w, nch], [1, ncol]])


def _feat_bias(t, nch):
    """DRAM [nch*128] -> SBUF [128, nch] feature-major bias."""
    return bass.AP(tensor=t, offset=0, ap=[[1, 128], [128, nch]])


def _bcast(t, n):
    """DRAM [n] -> [128, n] partition broadcast."""
    return bass.AP(tensor=t, offset=0, ap=[[0, 128], [1, n]])


def _rows(t, w, r0, nr):
    """DRAM [*, w] rows r0:r0+nr -> SBUF [nr, w]."""
    return bass.AP(tensor=t, offset=r0 * w, ap=[[w, nr], [1, w]])


def build_nc(reps: int = 1):
    nc = bacc.Bacc('TRN2', target_bir_lowering=False, debug=False,
                   num_devices=N_CORES)

    def din(name, shape, dt):
        return nc.dram_tensor(name, list(shape), dt, kind='ExternalInput')

    t = {n: din(n, s, dt) for n, s, dt in [
        ('qaT', [DA, TQ], BF16), ('qcT', [DC, TQ + 2], BF16),
        ('qres', [TQ, D], BF16),
        ('kT', [DA, TK], BF16), ('vT', [DA, TK], BF16),
        ('wqT', [DA, DA], BF16), ('wkT', [DA, DA], BF16),
        ('wvT', [DA, DA], BF16), ('wfT', [D, D], BF16),
        ('cw', [DC, 3], F32), ('bq', [DA], F32), ('bk', [DA], F32),
        ('bv', [DA], F32), ('gamma', [D], F32), ('beta', [D], F32)]}
    t['out'] = nc.dram_tensor('out', [TQ, D], F32, kind='ExternalOutput')

    with tile.TileContext(nc) as tc:
        for rep in range(reps):
            _build_rep(nc, tc, rep, t)
    nc.compile()
    return nc


def _build_rep(nc, tc, rep, t):
    R = f'r{rep}_'
    with tc.tile_pool(name=R + 'persist', bufs=1) as pp:
        fT_s = pp.tile([128, ND, TQ], BF16)          # concat features, fm
        q_s = pp.tile([128, NDA, TQ], BF16)
        k_s = pp.tile([128, NDA, TK], BF16)
        vaug = pp.tile([128, NKC, H, 66], BF16)
        o_u = pp.tile([128, NDA, TQ], BF16)          # unnormalized attn out
        r_all = pp.tile([1, H, TQ], BF16)            # per-head 1/denom

        # ---------------- input DMAs ----------------
        with tc.tile_pool(name=R + 'wts', bufs=1) as wp, \
             tc.tile_pool(name=R + 'xin', bufs=1) as xp, \
             tc.tile_pool(name=R + 'ps_proj', bufs=2, space='PSUM') as ps_p:
            qcT_s = xp.tile([128, NDA, TQ + 2], BF16)
            nc.sync.dma_start(out=qcT_s, in_=_chunked(t['qcT'], NDA, TQ + 2))
            xa_s = xp.tile([128, NDA, TQ], BF16)
            nc.sync.dma_start(out=xa_s, in_=_chunked(t['qaT'], NDA, TQ))
            xv_s = xp.tile([128, NDA, TK], BF16)
            nc.sync.dma_start(out=xv_s, in_=_chunked(t['vT'], NDA, TK))
            xk_s = xp.tile([128, NDA, TK], BF16)
            nc.sync.dma_start(out=xk_s, in_=_chunked(t['kT'], NDA, TK))
            cw_s = wp.tile([128, NDA, 3], F32)
            nc.sync.dma_start(out=cw_s, in_=_chunked(t['cw'], NDA, 3))
            wq_s = wp.tile([128, NDA, DA], BF16)
            nc.sync.dma_start(out=wq_s, in_=_chunked(t['wqT'], NDA, DA))
            wk_s = wp.tile([128, NDA, DA], BF16)
            nc.sync.dma_start(out=wk_s, in_=_chunked(t['wkT'], NDA, DA))
            wv_s = wp.tile([128, NDA, DA], BF16)
            nc.sync.dma_start(out=wv_s, in_=_chunked(t['wvT'], NDA, DA))
            bq_s = wp.tile([128, NDA], F32)
            nc.sync.dma_start(out=bq_s, in_=_feat_bias(t['bq'], NDA))
            bk_s = wp.tile([128, NDA], F32)
            nc.sync.dma_start(out=bk_s, in_=_feat_bias(t['bk'], NDA))
            bv_bc = wp.tile([128, DA], F32)
            nc.sync.dma_start(out=bv_bc, in_=_bcast(t['bv'], DA))

            # ---------------- conv branch (DVE only) ----------------
            for c in range(NDA):
                nc.vector.tensor_scalar_mul(
                    fT_s[:, NDA + c, :], qcT_s[:, c, 0:TQ], cw_s[:, c, 0:1])
                for k in (1, 2):
                    nc.vector.scalar_tensor_tensor(
                        fT_s[:, NDA + c, :], qcT_s[:, c, k:k + TQ],
                        cw_s[:, c, k:k + 1], fT_s[:, NDA + c, :],
                        AOp.mult, AOp.add)

            # ---------------- V proj (token-major, augmented ones) ------
            nc.vector.memset(vaug[:, :, :, 64:66], 1.0)
            for tt in range(NTK):
                for kk in range(4):
                    kc = tt * 4 + kk
                    pv = ps_p.tile([128, 512], F32, tag='projp')
                    for dc in range(NDA):
                        nc.tensor.matmul(
                            pv[:, :],
                            xv_s[:, dc, tt * 512 + kk * 128:
                                 tt * 512 + (kk + 1) * 128],
                            wv_s[:, dc, :],
                            start=(dc == 0), stop=(dc == NDA - 1))
                    nc.vector.tensor_tensor(
                        vaug[:, kc, :, 0:64],
                        pv[:].rearrange('p (h x) -> p h x', h=H),
                        bv_bc[:].rearrange('p (h x) -> p h x', h=H),
                        AOp.add)

            # ---------------- Q/K proj (feature-major) ----------------
            for tt in range(NTQ):
                for oc in range(NDA):
                    pq = ps_p.tile([128, 512], F32, tag='projp')
                    for dc in range(NDA):
                        nc.tensor.matmul(
                            pq[:, :],
                            wq_s[:, dc, oc * 128:(oc + 1) * 128],
                            xa_s[:, dc, tt * 512:(tt + 1) * 512],
                            start=(dc == 0), stop=(dc == NDA - 1))
                    nc.scalar.activation(
                        q_s[:, oc, tt * 512:(tt + 1) * 512], pq[:, :],
                        Identity, bias=bq_s[:, oc:oc + 1])
            for tt in range(NTK):
                for oc in range(NDA):
                    pk = ps_p.tile([128, 512], F32, tag='projp')
                    for dc in range(NDA):
                        nc.tensor.matmul(
                            pk[:, :],
                            wk_s[:, dc, oc * 128:(oc + 1) * 128],
                            xk_s[:, dc, tt * 512:(tt + 1) * 512],
                            start=(dc == 0), stop=(dc == NDA - 1))
                    nc.scalar.activation(
                        k_s[:, oc, tt * 512:(tt + 1) * 512], pk[:, :],
                        Identity, bias=bk_s[:, oc:oc + 1])

        # ---------------- attention ----------------
        with tc.tile_pool(name=R + 'pst', bufs=2, space='PSUM') as ps_s, \
             tc.tile_pool(name=R + 'pso', bufs=2, space='PSUM') as ps_o, \
             tc.tile_pool(name=R + 'pwork', bufs=3) as wp2:
            for h in range(H):
                hp = (h % 2) * 64
                hc = h // 2
                o_ps = ps_o.tile([128, TQ], F32, tag='o')
                for kc in range(NKC):
                    s_ps = ps_s.tile([128, TQ], F32, tag='s')
                    for tt in range(NTQ):
                        nc.tensor.matmul(
                            s_ps[:, tt * 512:(tt + 1) * 512],
                            k_s[hp:hp + 64, hc, kc * 128:(kc + 1) * 128],
                            q_s[hp:hp + 64, hc, tt * 512:(tt + 1) * 512],
                            start=True, stop=True, skip_group_check=True)
                    p_sb = wp2.tile([128, TQ], BF16, tag='p')
                    nc.scalar.activation(p_sb[:, :], s_ps[:, :], Exp,
                                         scale=0.125)
                    for tt in range(NTQ):
                        nc.tensor.matmul(
                            o_ps[0:65, tt * 512:(tt + 1) * 512],
                            vaug[:, kc, h, 0:65],
                            p_sb[:, tt * 512:(tt + 1) * 512],
                            start=(kc == 0), stop=(kc == NKC - 1),
                            skip_group_check=True)
                with nc.allow_low_precision(reason='bf16 1/denom, tol 2e-2'):
                    nc.vector.reciprocal(r_all[0:1, h, :], o_ps[64:65, :])
                nc.vector.tensor_copy(o_u[hp:hp + 64, hc, :], o_ps[0:64, :])

        # ---------------- normalize heads (one DRAM bounce) ----------
        with tc.tile_pool(name=R + 'dscr', bufs=1, space='DRAM') as dp, \
             tc.tile_pool(name=R + 'nrm', bufs=1) as np_:
            rscr = dp.tile([8, TQ], BF16, tag='rscr')
            nc.sync.dma_start(
                out=bass.AP(tensor=rscr.tensor, offset=rscr.offset,
                            ap=[[8 * TQ, 1], [1, 8 * TQ]]),
                in_=r_all[0:1, :, :])
            rb = np_.tile([128, NDA, TQ], BF16)
            for a in range(2):
                nc.sync.dma_start(
                    out=rb[a * 64:(a + 1) * 64, :, :],
                    in_=bass.AP(tensor=rscr.tensor,
                                offset=rscr.offset + a * TQ,
                                ap=[[0, 64], [2 * TQ, NDA], [1, TQ]]))
            for c in range(NDA):
                nc.vector.tensor_tensor(
                    fT_s[:, c, :], o_u[:, c, :], rb[:, c, :], AOp.mult)

        # ---------------- W_eff + residual + LayerNorm ----------------
        with tc.tile_pool(name=R + 'fin', bufs=1) as fp, \
             tc.tile_pool(name=R + 'lnw', bufs=3) as lp, \
             tc.tile_pool(name=R + 'ps_f', bufs=2, space='PSUM') as ps_f:
            wf_s = fp.tile([128, ND, D], BF16)
            nc.sync.dma_start(out=wf_s, in_=_chunked(t['wfT'], ND, D))
            ga_bc = fp.tile([128, D], F32)
            nc.sync.dma_start(out=ga_bc, in_=_bcast(t['gamma'], D))
            be_bc = fp.tile([128, D], F32)
            nc.sync.dma_start(out=be_bc, in_=_bcast(t['beta'], D))
            eps_s = fp.tile([128, 1], F32)
            nc.vector.memset(eps_s, LN_EPS)

            for i in range(NQC):
                pf = ps_f.tile([128, D], F32, tag='f')
                for ot in range(2):
                    for fc in range(ND):
                        nc.tensor.matmul(
                            pf[:, ot * 512:(ot + 1) * 512],
                            fT_s[:, fc, i * 128:(i + 1) * 128],
                            wf_s[:, fc, ot * 512:(ot + 1) * 512],
                            start=(fc == 0), stop=(fc == ND - 1))
                res = lp.tile([128, D], BF16, tag='res')
                nc.sync.dma_start(out=res, in_=_rows(t['qres'], D, i * 128, 128))
                x_s = lp.tile([128, D], F32, tag='x')
                nc.vector.scalar_tensor_tensor(x_s[:, :], pf[:, :], 1.0,
                                               res[:, :], AOp.mult, AOp.add)
                stats = lp.tile([128, 2, 6], F32, tag='st')
                nc.vector.bn_stats(stats[:, 0, :], x_s[:, 0:512])
                nc.vector.bn_stats(stats[:, 1, :], x_s[:, 512:1024])
                mv = lp.tile([128, 2], F32, tag='mv')
                nc.vector.bn_aggr(mv[:, :], stats[:, :, :])
                sd = lp.tile([128, 1], F32, tag='sd')
                nc.scalar.activation(sd[:, :], mv[:, 1:2], Sqrt,
                                     bias=eps_s[:, 0:1])
                rstd = lp.tile([128, 1], F32, tag='rs')
                nc.vector.reciprocal(rstd[:, :], sd[:, :])
                nm = lp.tile([128, 1], F32, tag='nm')
                nc.vector.scalar_tensor_tensor(nm[:, :], mv[:, 0:1], -1.0,
                                               rstd[:, :], AOp.mult, AOp.mult)
                t1 = lp.tile([128, D], F32, tag='t1')
                nc.scalar.activation(t1[:, :], x_s[:, :], Identity,
                                     bias=nm[:, 0:1], scale=rstd[:, 0:1])
                o_sb = lp.tile([128, D], F32, tag='ob')
                nc.any.tensor_tensor(o_sb[:, :], t1[:, :], ga_bc[:, :],
                                     AOp.mult)
                nc.any.tensor_tensor(o_sb[:, :], o_sb[:, :], be_bc[:, :],
                                     AOp.add)
                nc.sync.dma_start(out=_rows(t['out'], D, i * 128, 128),
                                  in_=o_sb[:, :])


def make_in_maps(inputs):
    f32 = np.float32
    q = np.ascontiguousarray(np.asarray(inputs['queries'], f32))
    k = np.ascontiguousarray(np.asarray(inputs['keys'], f32))
    v = np.ascontiguousarray(np.asarray(inputs['values'], f32))
    Wf = np.asarray(inputs['Wf'], f32)
    Woa = np.asarray(inputs['Wo_attn'], f32)
    Woc = np.asarray(inputs['Wo_conv'], f32)
    # torch Linear convention: y = x @ W.T + b with W (out, in)
    W_eff = np.concatenate([Wf[:, :DA] @ Woa, Wf[:, DA:] @ Woc], axis=1)
    b_eff = (np.asarray(inputs['bf'], f32)
             + Wf[:, :DA] @ np.asarray(inputs['bo_attn'], f32)
             + Wf[:, DA:] @ (Woc @ np.asarray(inputs['conv_b'], f32)
                             + np.asarray(inputs['bo_conv'], f32)))
    com = {
        'wqT': np.asarray(inputs['Wq'], f32).T.astype(NPBF),
        'wkT': np.asarray(inputs['Wk'], f32).T.astype(NPBF),
        'wvT': np.asarray(inputs['Wv'], f32).T.astype(NPBF),
        'wfT': W_eff.T.astype(NPBF),
        'cw': np.asarray(inputs['conv_w'], f32).reshape(DC, 3),
        'bq': np.asarray(inputs['bq'], f32),
        'bk': np.asarray(inputs['bk'], f32),
        'bv': np.asarray(inputs['bv'], f32),
        'gamma': np.asarray(inputs['gamma'], f32),
        'beta': np.asarray(inputs['beta'], f32),
    }
    com = {n: np.ascontiguousarray(a) for n, a in com.items()}
    in_maps = []
    for core in range(N_CORES):
        b, half = core // 2, core % 2
        r0, r1 = half * TQ, (half + 1) * TQ
        qc = np.zeros((TQ + 2, DC), f32)
        qc[1:TQ + 1] = q[b, r0:r1, DA:]
        if r0 > 0:
            qc[0] = q[b, r0 - 1, DA:]
        if r1 < L:
            qc[TQ + 1] = q[b, r1, DA:]
        m = dict(com)
        m['qaT'] = np.ascontiguousarray(q[b, r0:r1, :DA].T.astype(NPBF))
        m['qcT'] = np.ascontiguousarray(qc.T.astype(NPBF))
        m['qres'] = np.ascontiguousarray((q[b, r0:r1, :] + b_eff).astype(NPBF))
        m['kT'] = np.ascontiguousarray(k[b, :, :DA].T.astype(NPBF))
        m['vT'] = np.ascontiguousarray(v[b, :, :DA].T.astype(NPBF))
        in_maps.append(m)
    return in_maps


_NC_CACHE = {}


def get_nc(reps=1):
    if reps not in _NC_CACHE:
        _NC_CACHE[reps] = build_nc(reps)
    return _NC_CACHE[reps]


def kernel(**inputs):
    from concourse.bass_utils import run_bass_kernel_spmd
    nc = get_nc(1)
    in_maps = make_in_maps(inputs)
    res = run_bass_kernel_spmd(nc, in_maps, core_ids=list(range(N_CORES)))
    out = np.empty((B, L, D), np.float32)
    for core in range(N_CORES):
        b, half = core // 2, core % 2
        out[b, half * TQ:(half + 1) * TQ, :] = res.results[core]['out']
    return out
